# revision 15
# baseline (speedup 1.0000x reference)
"""TRN2 Bass kernel for nn_CardClassifier.

CNN(4x conv3x3+relu+maxpool2) -> per-feature sigmoid attention ->
128 stacked expert MLPs -> fusion MLP (2048->2038->2028->53).

Distribution: data-parallel convs (8 cores x 4 images), then 4 per-image
AllToAlls (overlapped under per-image conv3/4/attention chains that are
interleaved with the conv2 image pairs) to expert parallelism
(16 experts/core x 32 images), K-sharded fusion with two ReduceScatters;
final 53-dim partials summed on the host.

Engine balance: conv matmuls use tap-folded K packing; conv2 runs two
images concurrently via PE column tiling; maxpool/bias/relu evacuation
rotates between a DVE-reduce path and an ACT-evacuate + DVE 2-stage-max
path; conv3's third-ky operand is a shifted view of the ky0/ky1 replica
tile (weights parked at array rows 64-127).
"""

import sys

sys.path.insert(0, "/opt/trn_rl_repo")

import json as _json
import contextlib
import numpy as np
import ml_dtypes

import bass_rust
import concourse.bass as bass
import concourse.mybir as mybir
from concourse import tile
from concourse.bass_utils import run_bass_kernel_spmd

F32 = mybir.dt.float32
BF16 = mybir.dt.bfloat16
AF = mybir.ActivationFunctionType
ALU = None  # filled lazily
BF = ml_dtypes.bfloat16

B, H, W = 32, 224, 224
NCORES, BL = 8, 4
NF, FLAT = 128, 196
EXP_DIMS = [196, 196, 196, 98, 24, 16]
FIN = [2048, 2038, 2028, 53]
EPC = 16  # experts per core

_BUILT = None
RG = [list(range(NCORES))]

# ---- const pack layouts: name -> (row0, rows, col_off, cols) ----
WPK_LAYOUT = {}
_off = 0
for _name, _r0, _rows, _cols in (
    ("cw1p", 0, 108, 128), ("cw2p", 0, 96, 192), ("cw3ap", 0, 128, 384),
    ("cw3bp", 64, 64, 384), ("cw4p", 0, 128, 1152), ("awr", 0, 128, FLAT),
):
    WPK_LAYOUT[_name] = (_r0, _rows, _off, _cols)
    _off += _cols
WPK_COLS = _off

EPK_LAYOUT = {}
_off = 0
for _name, _rows, _cols in (
    ("e1AA", 128, EPC * 128), ("e1AB", 128, EPC * 68),
    ("e1BA", 69, EPC * 128), ("e1BB", 69, EPC * 68),
    ("e2AA", 128, EPC * 128), ("e2AB", 128, EPC * 68),
    ("e2BA", 69, EPC * 128), ("e2BB", 69, EPC * 68),
    ("e3A", 128, EPC * 98), ("e3B", 69, EPC * 98),
    ("e4", 99, EPC * 24), ("e5p", 64, 256),
    ("fw1s", 128, 4096), ("fw2s", 128, 4096), ("fw3s", 128, 106),
    ("fb1d8", 1, 2048), ("fb2d8", 1, 2048), ("fb3d8", 1, 53),
    ("onesr", 1, 512), ("t5f", 64, 256),
):
    EPK_LAYOUT[_name] = (_rows, _off, _cols)
    _off += _cols
EPK_COLS = (_off + 15) // 16 * 16
# staged sub-loads of the expert pack (col ranges; e1 first so the first
# expert layer can start while the rest still streams in)
EPK_SPLITS = [0, 6272, 12544, 16320, EPK_COLS]


# ---------------------------------------------------------------- tilefix
def _fix_bir_json(raw: bytes) -> bytes:
    """This walrus build allows at most 1 sync-wait per instruction; Tile's
    tail drain can carry more. Split extras onto NoOp carriers."""
    d = _json.loads(raw)
    k = 0
    for fn in d.get("functions", []):
        for blk in fn.get("blocks", []):
            out = []
            for inst in blk["instructions"]:
                si = inst.get("sync_info")
                waits = (si or {}).get("on_wait") or []
                if len(waits) > 1:
                    for wchunk in waits[:-1]:
                        out.append({
                            "debug": inst.get("debug", 0),
                            "engine": inst["engine"],
                            "ins": [], "outs": [],
                            "name": f"NOPW-{k}",
                            "opcode": "NoOp",
                            "sync_info": {"on_update": [], "on_wait": [wchunk]},
                        })
                        k += 1
                    si["on_wait"] = waits[-1:]
                out.append(inst)
            blk["instructions"] = out
    return _json.dumps(d).encode()


def _vp(dims):
    return bass_rust.VecI64Pair(dims)


# ---------------------------------------------------------------- build
def _build():
    global ALU
    from concourse.alu_op_type import AluOpType as ALU_

    ALU = ALU_
    nc = bass.Bass("TRN2", target_bir_lowering=False, debug=False,
                   num_devices=NCORES)

    dp = lambda name, shape, dt: nc.declare_dram_parameter(name, list(shape), dt, isOutput=False)

    xp = dp("xpad9", [108 * 226 * 226], BF16)  # host 9-tap im2col replicas
    wpk_in = dp("wpk", [128, WPK_COLS], BF16)
    wpkf_in = dp("wpkf", [128, 8], F32)
    epk_in = dp("epk", [128, EPK_COLS], BF16)

    y_out = nc.declare_dram_parameter("y", [32, 53], F32, isOutput=True)

    with tile.TileContext(nc, pool_alloc_mode="queue") as tc:
        stk = contextlib.ExitStack()
        with stk:
            # ---- persistent consts (two batched DMAs) --------------------
            wpool = stk.enter_context(tc.tile_pool(name="wconst", bufs=1))
            wsb = wpool.tile([128, WPK_COLS], BF16)
            nc.scalar.dma_start(wsb[:], wpk_in[:])
            wfsb = wpool.tile([128, 8], F32)
            nc.scalar.dma_start(wfsb[:], wpkf_in[:])

            def wv(name):
                r0, r, o, c = WPK_LAYOUT[name]
                return wsb[r0:r0 + r, o:o + c]

            cw1 = wv("cw1p")
            cw2 = wv("cw2p")
            cw3a = wv("cw3ap")
            cw3b = wv("cw3bp")   # parked at SBUF partitions 64..127
            cw4 = wv("cw4p")
            awsb = wv("awr")
            cb1 = wfsb[:, 0:1]
            cb2p = wfsb[:, 1:2]   # cb2 tiled x2 for image-pair packing
            cb3 = wfsb[:, 2:3]
            cb4 = wfsb[:, 3:4]
            absb = wfsb[:, 4:5]
            ones1 = wpool.tile([1, 32], BF16)
            nc.vector.memset(ones1[:], 1.0)

            # conv2->conv3 replicas: [p0:64]=ky0 shifted, [p64:128]=ky1;
            # conv3's ky2 operand is the ky1 half viewed one row further.
            a2pool = stk.enter_context(tc.tile_pool(name="a2r", bufs=1))
            A2R = [a2pool.tile([128, 57 * 58], BF16, name=f"a2a{i}")
                   for i in range(BL)]

            a3pool = stk.enter_context(tc.tile_pool(name="a3m", bufs=1))
            A3 = [a3pool.tile([128, 30 * 30], BF16, name=f"a3_{i}")
                  for i in range(BL)]

            hpool = stk.enter_context(tc.tile_pool(name="hp", bufs=1))
            Ht = [hpool.tile([128, FLAT], BF16, name=f"ht{i}") for i in range(BL)]
            HB = [hpool.tile([128, 256], BF16, name=f"hb{i}") for i in range(BL)]
            for i in range(BL):
                nc.vector.memset(HB[i][:, FLAT:256], 0.0)

            tpool = stk.enter_context(tc.tile_pool(name="texp", bufs=1))
            TA1 = tpool.tile([128, 512], BF16)
            TB1 = tpool.tile([69, 512], BF16)

            dram = stk.enter_context(tc.tile_pool(name="dram", bufs=1, space="DRAM"))
            in_bs = [dram.tile([128, 256], BF16, name=f"a2ain{i}") for i in range(BL)]
            out_bs = [dram.tile([128, 256], BF16, name=f"a2aout{i}") for i in range(BL)]
            rs1i = dram.tile([2048, 32], BF16)
            rs1o = dram.tile([256, 32], BF16)
            rs2i = dram.tile([2048, 32], BF16)
            rs2o = dram.tile([256, 32], BF16)

            a1stk = contextlib.ExitStack()
            a1pool = a1stk.enter_context(tc.tile_pool(name="a1r", bufs=1))
            A1R = [a1pool.tile([96, 112 * 114], BF16, name=f"a1r{i}")
                   for i in range(BL)]
            a1vs = [A1R[i].rearrange("p (r c) -> p r c", c=114) for i in range(BL)]
            for i in range(BL):
                nc.vector.memset(a1vs[i][0:32, 0, :], 0.0)
                nc.vector.memset(a1vs[i][64:96, 111, :], 0.0)

            pmstk = contextlib.ExitStack()
            pmpool = pmstk.enter_context(tc.tile_pool(name="c1pm", bufs=1))
            pm = pmpool.tile([128, 32 * 114], BF16)  # rolling 32 pooled rows
            pmv = pm.rearrange("p (r c) -> p r c", c=114)
            nc.vector.memset(pmv[:, :, 0], 0.0)
            nc.vector.memset(pmv[:, :, 113], 0.0)

            # =========================================================
            # conv1: 3->32, im2col K=108 (9 taps x 4img x 3ch), M=128
            # 14 strips of 16 conv rows; PSUM groups of 2 MMs (2 banks,
            # 2 conv rows each, 256-col padded lanes). Per group rotate:
            #   A: DVE reduce + DVE bias/relu
            #   B: ACT evac(bias+relu,bf16) + DVE 2-stage max (2x vertical)
            # pm is a rolling 32-row window; A1R replicas built per 2 strips.
            # =========================================================
            gctr = {"n": 0}
            PAT1 = ["B", "B", "B", "B", "A"]

            with tc.tile_pool(name="c1x", bufs=2) as xpool, \
                 tc.tile_pool(name="c1e", bufs=4) as epool, \
                 tc.tile_pool(name="c1v", bufs=6) as vpool, \
                 tc.tile_pool(name="ps1", bufs=4, space="PSUM") as psc:

                def drain1(P, rows0, path):
                    # P [128,1024] = (g2, v2, c256pad); -> 2 pooled pm rows
                    out = pmv[:, rows0:rows0 + 2, 1:113]
                    if path == "A":
                        tmp = vpool.tile([128, 224], F32, tag="t1a")
                        tv = tmp.rearrange("p (g c) -> p g c", g=2)
                        Pv = P.rearrange("p (g v ch t) -> p g ch v t",
                                         g=2, v=2, ch=128, t=2)
                        nc.vector.tensor_reduce(tv[:], Pv[:, :, 0:112, :, :],
                                                axis=mybir.AxisListType.XY,
                                                op=ALU.max)
                        nc.vector.tensor_scalar(out, tv[:], cb1[:, 0:1], 0.0,
                                                op0=ALU.add, op1=ALU.max)
                    else:
                        eb = epool.tile([128, 896], BF16, tag="t1e")
                        ebv = eb.rearrange("p (g v c) -> p g v c", g=2, v=2, c=224)
                        Pv = P.rearrange("p (g v c) -> p g v c", g=2, v=2, c=256)
                        nc.scalar.activation(ebv[:], Pv[:, :, :, 0:224], AF.Relu,
                                             bias=cb1[:, 0:1])
                        v1 = vpool.tile([128, 448], BF16, tag="t1v")
                        v1v = v1.rearrange("p (g c) -> p g c", g=2)
                        nc.vector.tensor_tensor(v1v[:], ebv[:, :, 0, :],
                                                ebv[:, :, 1, :], op=ALU.max)
                        v1h = v1.rearrange("p (g c t) -> p g c t", g=2, c=112, t=2)
                        nc.vector.tensor_tensor(out, v1h[:, :, :, 0],
                                                v1h[:, :, :, 1], op=ALU.max)

                for s in range(14):
                    r0 = 16 * s
                    X9 = xpool.tile([108, 16 * 226], BF16, tag="x9")
                    src = xp[:]
                    src.ap = _vp([[51076, 108], [1, 3616]])
                    src.offset = src.offset + r0 * 226
                    nc.sync.dma_start(X9[:], src)
                    X9v = X9.rearrange("p (r c) -> p r c", c=226)
                    for gp in range(4):
                        P = psc.tile([128, 1024], F32, tag="acc")
                        Pmm = P.rearrange("p (g v c) -> p g v c", g=2, v=2, c=256)
                        for j in range(2):
                            rr = 4 * gp + 2 * j
                            nc.tensor.matmul(Pmm[:, j, :, 0:224], cw1[:],
                                             X9v[:, rr:rr + 2, 0:224],
                                             start=True, stop=True)
                        drain1(P, (8 * s + 2 * gp) % 32,
                               PAT1[gctr["n"] % 5])
                        gctr["n"] += 1
                    # replicate pooled rows into A1R every 2 strips
                    if s % 2 == 1:
                        k = s // 2
                        R0, R1 = 16 * k, 16 * k + 16
                        W0 = R0 % 32
                        for i in range(BL):
                            sp = pmv[32 * i:32 * i + 32, :, :]
                            av = a1vs[i]
                            q = nc.sync if i < 2 else nc.gpsimd
                            q.dma_start(av[32:64, R0:R1, :],
                                        sp[:, W0:W0 + 16, :])
                            s1 = min(R1, 111)
                            q.dma_start(av[0:32, R0 + 1:s1 + 1, :],
                                        sp[:, W0:W0 + (s1 - R0), :])
                            s0 = max(R0, 1)
                            q.dma_start(av[64:96, s0 - 1:R1 - 1, :],
                                        sp[:, W0 + (s0 - R0):W0 + 16, :])

            pmstk.close()

            # =========================================================
            # conv2: 32->64, K=96 (3ky x 32ch), 3 kx passes; TWO images
            # concurrently via PE column tiling (img a -> parts 0:64,
            # img b -> 64:128). PSUM groups of 2 t (2 banks); per t:
            # 4 conv rows x 112 (r2, v2, c128pad) -> 2 pooled rows x 56.
            # conv3+conv4+attention+AllToAll chains per image are
            # interleaved between the two conv2 pairs.
            # =========================================================
            PAT2 = ["B", "B", "B", "B", "A"]
            PAT34 = ["A", "B"]
            g2ctr = {"n": 0}
            rot = {"i": 0}

            with tc.tile_pool(name="c2pm", bufs=1) as pm2pool, \
                 tc.tile_pool(name="c2e", bufs=4) as e2pool, \
                 tc.tile_pool(name="c2v", bufs=4) as v2pool, \
                 tc.tile_pool(name="c3e", bufs=4) as e3pool, \
                 tc.tile_pool(name="c3v", bufs=4) as v3pool, \
                 tc.tile_pool(name="att", bufs=2) as atp, \
                 tc.tile_pool(name="ps2", bufs=3, space="PSUM") as psc2, \
                 tc.tile_pool(name="ps3", bufs=2, space="PSUM") as psc3:

                def conv2_pair(pr):
                    ia, ib = 2 * pr, 2 * pr + 1
                    pm2 = pm2pool.tile([128, 56 * 58], BF16, name=f"pm2_{pr}")
                    pm2v = pm2.rearrange("p (r c) -> p r c", c=58)
                    nc.vector.memset(pm2v[:, :, 0], 0.0)
                    nc.vector.memset(pm2v[:, :, 57], 0.0)

                    def drain2(P, G, path):
                        # P [128,1024] = (q4=t2*r2, v2, c128pad) -> 4 rows
                        out = pm2.rearrange("p (q c) -> p q c", c=58)[
                            :, 4 * G:4 * G + 4, 1:57]
                        if path == "A":
                            tmp = v2pool.tile([128, 224], F32, tag="t2a")
                            tv = tmp.rearrange("p (q c) -> p q c", q=4)
                            Pv = P.rearrange("p (q v ch hb) -> p q ch v hb",
                                             q=4, v=2, ch=64, hb=2)
                            nc.vector.tensor_reduce(tv[:], Pv[:, :, 0:56, :, :],
                                                    axis=mybir.AxisListType.XY,
                                                    op=ALU.max)
                            nc.vector.tensor_scalar(out, tv[:], cb2p[:, 0:1],
                                                    0.0, op0=ALU.add,
                                                    op1=ALU.max)
                        else:
                            eb = e2pool.tile([128, 896], BF16, tag="t2e")
                            ebv = eb.rearrange("p (q v c) -> p q v c",
                                               q=4, v=2, c=112)
                            Pv = P.rearrange("p (q v c) -> p q v c",
                                             q=4, v=2, c=128)
                            nc.scalar.activation(ebv[:], Pv[:, :, :, 0:112],
                                                 AF.Relu, bias=cb2p[:, 0:1])
                            v1 = v2pool.tile([128, 448], BF16, tag="t2v")
                            v1v = v1.rearrange("p (q c) -> p q c", q=4)
                            nc.vector.tensor_tensor(v1v[:], ebv[:, :, 0, :],
                                                    ebv[:, :, 1, :], op=ALU.max)
                            v1h = v1.rearrange("p (q c t) -> p q c t",
                                               q=4, c=56, t=2)
                            nc.vector.tensor_tensor(out, v1h[:, :, :, 0],
                                                    v1h[:, :, :, 1], op=ALU.max)

                    rha = A1R[ia].rearrange("p (q v c) -> p q v c",
                                            q=56, v=2, c=114)
                    rhb = A1R[ib].rearrange("p (q v c) -> p q v c",
                                            q=56, v=2, c=114)
                    for G in range(14):
                        P = psc2.tile([128, 1024], F32, tag="acc2")
                        Pmm = P.rearrange("p (t r v c) -> p t r v c",
                                          t=2, r=2, v=2, c=128)
                        for j in range(2):
                            t = 2 * G + j
                            for kx in range(3):
                                nc.tensor.matmul(
                                    Pmm[0:64, j, :, :, 0:112],
                                    cw2[:, 64 * kx:64 * kx + 64],
                                    rha[:, 2 * t:2 * t + 2, :, kx:kx + 112],
                                    start=(kx == 0), stop=(kx == 2),
                                    tile_position=(0, 0))
                                nc.tensor.matmul(
                                    Pmm[64:128, j, :, :, 0:112],
                                    cw2[:, 64 * kx:64 * kx + 64],
                                    rhb[:, 2 * t:2 * t + 2, :, kx:kx + 112],
                                    start=(kx == 0), stop=(kx == 2),
                                    tile_position=(0, 64))
                        drain2(P, G, PAT2[g2ctr["n"] % 5])
                        g2ctr["n"] += 1
                    # A2R replicas (ky0 shifted into parts 0:64, ky1 into
                    # 64:128 with zero row 56) for both images of the pair
                    for j, img in enumerate((ia, ib)):
                        pv = pm2v[64 * j:64 * j + 64, :, :]
                        aav = A2R[img].rearrange("p (r c) -> p r c", c=58)
                        nc.vector.memset(aav[0:64, 0, :], 0.0)
                        nc.vector.memset(aav[64:128, 56, :], 0.0)
                        q = (nc.sync, nc.gpsimd)[j]
                        q.dma_start(aav[0:64, 1:57, :], pv)
                        q.dma_start(aav[64:128, 0:56, :], pv)

                def drain3(P, a3v, t, path):
                    # P [128,448] = 8 conv rows x 56 -> 4 pooled rows x 28
                    out = a3v[:, 1 + 4 * t:1 + 4 * t + 4, 1:29]
                    if path == "A":
                        tmp = v3pool.tile([128, 112], F32, tag="t3a")
                        tv = tmp.rearrange("p (r c) -> p r c", c=28)
                        nc.vector.tensor_reduce(
                            tv[:],
                            P.rearrange("p (r a c b) -> p r c a b",
                                        r=4, a=2, c=28, b=2),
                            axis=mybir.AxisListType.XY, op=ALU.max)
                        nc.vector.tensor_scalar(out, tv[:], cb3[:, 0:1], 0.0,
                                                op0=ALU.add, op1=ALU.max)
                    else:
                        eb = e3pool.tile([128, 448], BF16, tag="t3e")
                        nc.scalar.activation(eb[:], P[:], AF.Relu, bias=cb3[:, 0:1])
                        ebv = eb.rearrange("p (r v c) -> p r v c", r=4, v=2, c=56)
                        v1 = v3pool.tile([128, 224], BF16, tag="t3v")
                        v1v = v1.rearrange("p (r c) -> p r c", c=56)
                        nc.vector.tensor_tensor(v1v[:], ebv[:, :, 0, :],
                                                ebv[:, :, 1, :], op=ALU.max)
                        v1h = v1.rearrange("p (r c t) -> p r c t", r=4, c=28, t=2)
                        nc.vector.tensor_tensor(out, v1h[:, :, :, 0],
                                                v1h[:, :, :, 1], op=ALU.max)

                def drain4(P, i, rb, nr, path):
                    # P [128, nr*28] -> pooled nr/2 x 14 into Ht[i]
                    htv = Ht[i].rearrange("p (r c) -> p r c", c=14)
                    out = htv[:, rb // 2:rb // 2 + nr // 2, :]
                    if path == "A":
                        tmp = v3pool.tile([128, 112], F32, tag="t3a")
                        tv = tmp.rearrange("p (r c) -> p r c", c=14)
                        nc.vector.tensor_reduce(
                            tv[:, 0:nr // 2, :],
                            P.rearrange("p (r a c b) -> p r c a b",
                                        r=nr // 2, a=2, c=14, b=2),
                            axis=mybir.AxisListType.XY, op=ALU.max)
                        nc.vector.tensor_scalar(out, tv[:, 0:nr // 2, :],
                                                cb4[:, 0:1], 0.0,
                                                op0=ALU.add, op1=ALU.max)
                    else:
                        eb = e3pool.tile([128, 448], BF16, tag="t3e")
                        nc.scalar.activation(eb[0:128, 0:nr * 28], P[:], AF.Relu,
                                             bias=cb4[:, 0:1])
                        ebv = eb.rearrange("p (r v c) -> p r v c", r=8, v=2, c=28)
                        v1 = v3pool.tile([128, 224], BF16, tag="t3v")
                        v1v = v1.rearrange("p (r c) -> p r c", c=28)
                        nc.vector.tensor_tensor(v1v[:, 0:nr // 2, :],
                                                ebv[:, 0:nr // 2, 0, :],
                                                ebv[:, 0:nr // 2, 1, :],
                                                op=ALU.max)
                        v1h = v1.rearrange("p (r c t) -> p r c t", r=8, c=14, t=2)
                        nc.vector.tensor_tensor(out, v1h[:, 0:nr // 2, :, 0],
                                                v1h[:, 0:nr // 2, :, 1],
                                                op=ALU.max)

                def chain(i):
                    # conv3 + conv4 + attention + AllToAll for image i
                    aav = A2R[i].rearrange("p (r c) -> p r c", c=58)
                    a3v = A3[i].rearrange("p (r c) -> p r c", c=30)
                    nc.vector.memset(a3v[:, 0, :], 0.0)
                    nc.vector.memset(a3v[:, 29, :], 0.0)
                    nc.vector.memset(a3v[:, :, 0], 0.0)
                    nc.vector.memset(a3v[:, :, 29], 0.0)
                    for t in range(7):
                        P = psc3.tile([128, 448], F32, tag="acc3")
                        for kx in range(3):
                            nc.tensor.matmul(P[:], cw3a[:, 128 * kx:128 * kx + 128],
                                             aav[:, 8 * t:8 * t + 8, kx:kx + 56],
                                             start=(kx == 0), stop=False)
                            nc.tensor.matmul(P[:], cw3b[:, 128 * kx:128 * kx + 128],
                                             aav[64:128, 8 * t + 1:8 * t + 9,
                                                 kx:kx + 56],
                                             start=False, stop=(kx == 2))
                        drain3(P, a3v, t, PAT34[(rot["i"] + t) % 2])
                    rot["i"] += 1

                    # conv4: K=128, 9 taps via views
                    for si, (rb, nr) in enumerate(((0, 16), (16, 12))):
                        P = psc3.tile([128, nr * 28], F32, tag="acc3")
                        for k in range(9):
                            ky, kx = divmod(k, 3)
                            nc.tensor.matmul(P[:], cw4[:, 128 * k:128 * k + 128],
                                             a3v[:, rb + ky:rb + ky + nr, kx:kx + 28],
                                             start=(k == 0), stop=(k == 8))
                        drain4(P, i, rb, nr, PAT34[(rot["i"] + si) % 2])

                    # attention: att = sigmoid(feats . aw + ab); h = feats*att
                    tmp = atp.tile([128, FLAT], F32, tag="tmp")
                    nc.vector.tensor_tensor(tmp[:], Ht[i][:], awsb[:], op=ALU.mult)
                    attv = atp.tile([128, 1], F32, tag="av")
                    nc.vector.tensor_reduce(attv[:], tmp[:],
                                            axis=mybir.AxisListType.X, op=ALU.add)
                    atts = atp.tile([128, 1], F32, tag="as")
                    nc.scalar.activation(atts[:], attv[:], AF.Sigmoid,
                                         bias=absb[:, 0:1])
                    nc.vector.tensor_scalar(HB[i][:, 0:FLAT], Ht[i][:],
                                            atts[:, 0:1], None, op0=ALU.mult)

                    # per-image AllToAll + transpose + relabel into TA1/TB1
                    nc.sync.dma_start(in_bs[i][:], HB[i][:])
                    nc.gpsimd.collective_compute(
                        "AllToAll", mybir.AluOpType.bypass, replica_groups=RG,
                        ins=[in_bs[i].opt()], outs=[out_bs[i].opt()])
                    TAr = atp.tile([128, 128], BF16, tag="tar")
                    TBr = atp.tile([128, 128], BF16, tag="tbr")
                    for blk, dst in ((0, TAr), (1, TBr)):
                        srcT = out_bs[i][:]
                        srcT.ap = _vp([[256, 128], [1, 128]])
                        srcT.offset = srcT.offset + 128 * blk
                        nc.sync.dma_start_transpose(dst[:], srcT)
                    dstA = TA1.rearrange("p (e s q) -> p e s q", e=EPC, s=8, q=4)
                    srcA = TAr.rearrange("p (s e q) -> p e s q", s=8, e=EPC, q=1)
                    nc.vector.tensor_copy(dstA[:, :, :, i:i + 1], srcA[:])
                    dstB = TB1.rearrange("p (e s q) -> p e s q", e=EPC, s=8, q=4)
                    srcB = TBr.rearrange("p (s e q) -> p e s q", s=8, e=EPC, q=1)
                    nc.vector.tensor_copy(dstB[0:68, :, :, i:i + 1], srcB[0:68])

                conv2_pair(0)
                chain(0)
                chain(1)
                conv2_pair(1)
                chain(2)
                chain(3)

            a1stk.close()

            # ---- expert + fusion weights: staged DMAs (land during the
            # tail of the image chains / first expert layer)
            ewfpool = stk.enter_context(tc.tile_pool(name="ewf", bufs=1))
            epk = ewfpool.tile([128, EPK_COLS], BF16)
            for si in range(len(EPK_SPLITS) - 1):
                o0, o1 = EPK_SPLITS[si], EPK_SPLITS[si + 1]
                q = (nc.sync, nc.gpsimd)[si % 2]
                q.dma_start(epk[:, o0:o1], epk_in[:, o0:o1])

            def ev(name):
                r, o, c = EPK_LAYOUT[name]
                return epk[0:r, o:o + c]

            nc.sync.dma_start(TB1[68:69, :], ev("onesr"))

            # =========================================================
            # experts: 16 local experts x 32 imgs, weight-stationary
            # =========================================================
            pse = stk.enter_context(tc.tile_pool(name="pse", bufs=5, space="PSUM"))
            pse2 = stk.enter_context(tc.tile_pool(name="pse2", bufs=2, space="PSUM"))

            def elayer(TAi, TBi, pre):
                PA = pse.tile([128, 512], F32, tag="pacc")
                PB = pse.tile([68, 512], F32, tag="pacc")
                wAA, wAB = ev(pre + "AA"), ev(pre + "AB")
                wBA, wBB = ev(pre + "BA"), ev(pre + "BB")
                for e in range(EPC):
                    sl = slice(32 * e, 32 * e + 32)
                    nc.tensor.matmul(PA[:, sl], wAA[:, 128 * e:128 * e + 128],
                                     TAi[:, sl], start=True, stop=False)
                    nc.tensor.matmul(PA[:, sl], wBA[:, 128 * e:128 * e + 128],
                                     TBi[:, sl], start=False, stop=True)
                    nc.tensor.matmul(PB[:, sl], wAB[:, 68 * e:68 * e + 68],
                                     TAi[:, sl], start=True, stop=False)
                    nc.tensor.matmul(PB[:, sl], wBB[:, 68 * e:68 * e + 68],
                                     TBi[:, sl], start=False, stop=True)
                TAo = tpool.tile([128, 512], BF16, name=pre + "oa")
                TBo = tpool.tile([69, 512], BF16, name=pre + "ob")
                nc.scalar.activation(TAo[:], PA[:], AF.Relu)
                nc.scalar.activation(TBo[0:68, :], PB[:], AF.Relu)
                nc.sync.dma_start(TBo[68:69, :], ev("onesr"))
                return TAo, TBo

            TA2, TB2 = elayer(TA1, TB1, "e1")
            TA3, TB3 = elayer(TA2, TB2, "e2")

            P98 = pse.tile([98, 512], F32, tag="pacc")
            for e in range(EPC):
                sl = slice(32 * e, 32 * e + 32)
                nc.tensor.matmul(P98[:, sl], ev("e3A")[:, 98 * e:98 * e + 98],
                                 TA3[:, sl], start=True, stop=False)
                nc.tensor.matmul(P98[:, sl], ev("e3B")[:, 98 * e:98 * e + 98],
                                 TB3[:, sl], start=False, stop=True)
            T4 = tpool.tile([99, 512], BF16)
            nc.scalar.activation(T4[0:98, :], P98[:], AF.Relu)
            nc.sync.dma_start(T4[98:99, :], ev("onesr"))

            P24 = pse.tile([24, 512], F32, tag="pacc")
            for e in range(EPC):
                sl = slice(32 * e, 32 * e + 32)
                nc.tensor.matmul(P24[:, sl], ev("e4")[:, 24 * e:24 * e + 24],
                                 T4[:, sl], start=True, stop=True)

            # pair layout for L5: rows 0..24 even expert, 32..56 odd
            T5R = tpool.tile([64, 256], BF16)
            nc.sync.dma_start(T5R[:], ev("t5f"))
            P24v = P24.rearrange("p (e g) -> p e g", g=32)
            T5Rv = T5R.rearrange("p (q g) -> p q g", g=32)
            nc.scalar.activation(T5Rv[0:24, :, :], P24v[0:24, 0:16:2, :], AF.Relu)
            nc.scalar.activation(T5Rv[32:56, :, :], P24v[0:24, 1:16:2, :], AF.Relu)

            S5 = pse2.tile([128, 64], F32, tag="ps5")
            for p in range(8):
                nc.tensor.matmul(S5[32 * (p % 4):32 * (p % 4) + 32,
                                    32 * (p // 4):32 * (p // 4) + 32],
                                 ev("e5p")[:, 32 * p:32 * p + 32],
                                 T5R[:, 32 * p:32 * p + 32],
                                 start=True, stop=True,
                                 tile_position=(0, 32 * (p % 4)))
            SF = tpool.tile([128, 64], BF16)
            nc.scalar.activation(SF[:], S5[:], AF.Relu)

            # =========================================================
            # fusion: K-sharded partials + ReduceScatter x2, host sum
            # =========================================================
            fw1sb, fw2sb, fw3sb = ev("fw1s"), ev("fw2s"), ev("fw3s")
            fb1sb, fb2sb, fb3sb = ev("fb1d8"), ev("fb2d8"), ev("fb3d8")

            P1 = pse.tile([128, 512], F32, tag="pacc")
            for mc in range(16):
                msl = slice(32 * mc, 32 * mc + 32)
                for g in range(2):
                    nc.tensor.matmul(P1[:, msl],
                                     fw1sb[:, 2048 * g + 128 * mc:2048 * g + 128 * mc + 128],
                                     SF[:, 32 * g:32 * g + 32],
                                     start=(g == 0), stop=False)
                nc.tensor.matmul(P1[:, msl], fb1sb[:, 128 * mc:128 * mc + 128],
                                 ones1[:], start=False, stop=True)
            S1pre = tpool.tile([128, 512], BF16)
            nc.scalar.activation(S1pre[:], P1[:], AF.Copy)
            d1 = rs1i[:]
            d1.ap = _vp([[32, 128], [4096, 16], [1, 32]])
            nc.sync.dma_start(d1, S1pre[:])
            nc.gpsimd.collective_compute(
                "ReduceScatter", mybir.AluOpType.add, replica_groups=RG,
                ins=[rs1i.opt()], outs=[rs1o.opt()])
            S1c = tpool.tile([128, 64], BF16)
            sr = rs1o[:]
            sr.ap = _vp([[32, 128], [4096, 2], [1, 32]])
            nc.sync.dma_start(S1c[:], sr)
            S1 = tpool.tile([128, 64], BF16)
            nc.scalar.activation(S1[:], S1c[:], AF.Relu)

            P2 = pse.tile([128, 512], F32, tag="pacc")
            for mc in range(16):
                msl = slice(32 * mc, 32 * mc + 32)
                for kc in range(2):
                    nc.tensor.matmul(P2[:, msl],
                                     fw2sb[:, 2048 * kc + 128 * mc:2048 * kc + 128 * mc + 128],
                                     S1[:, 32 * kc:32 * kc + 32],
                                     start=(kc == 0), stop=False)
                nc.tensor.matmul(P2[:, msl], fb2sb[:, 128 * mc:128 * mc + 128],
                                 ones1[:], start=False, stop=True)
            S2pre = tpool.tile([128, 512], BF16)
            nc.scalar.activation(S2pre[:], P2[:], AF.Copy)
            d2 = rs2i[:]
            d2.ap = _vp([[32, 128], [4096, 16], [1, 32]])
            nc.sync.dma_start(d2, S2pre[:])
            nc.gpsimd.collective_compute(
                "ReduceScatter", mybir.AluOpType.add, replica_groups=RG,
                ins=[rs2i.opt()], outs=[rs2o.opt()])
            S2c = tpool.tile([128, 64], BF16)
            sr2 = rs2o[:]
            sr2.ap = _vp([[32, 128], [4096, 2], [1, 32]])
            nc.sync.dma_start(S2c[:], sr2)
            S2 = tpool.tile([128, 64], BF16)
            nc.scalar.activation(S2[:], S2c[:], AF.Relu)

            P3 = pse2.tile([53, 32], F32, tag="ps5")
            for kc in range(2):
                nc.tensor.matmul(P3[:], fw3sb[:, 53 * kc:53 * kc + 53],
                                 S2[:, 32 * kc:32 * kc + 32],
                                 start=(kc == 0), stop=False)
            nc.tensor.matmul(P3[:], fb3sb[:], ones1[:], start=False, stop=True)
            S3 = tpool.tile([53, 32], F32)
            nc.scalar.activation(S3[:], P3[:], AF.Copy)
            nc.sync.dma_start(y_out[:].rearrange("b o -> o b"), S3[:])

    orig = nc.to_json_bytes
    nc.to_json_bytes = lambda: _fix_bir_json(orig())
    return nc


# ---------------------------------------------------------------- host prep
def _host_shared(inputs):
    f32 = np.float32
    cw = [np.asarray(inputs[f"cw{i+1}"], f32) for i in range(4)]
    cb = [np.asarray(inputs[f"cb{i+1}"], f32) for i in range(4)]

    wpk = np.zeros((128, WPK_COLS), f32)

    def put(name, arr):
        r0, r, o, c = WPK_LAYOUT[name]
        assert arr.shape == (r, c), (name, arr.shape)
        wpk[r0:r0 + r, o:o + c] = arr

    t = np.zeros((108, 128), f32)
    for ky in range(3):
        for kx in range(3):
            blk = cw[0][:, :, ky, kx].T
            for img in range(4):
                r = (ky * 3 + kx) * 12 + img * 3
                t[r:r + 3, img * 32:(img + 1) * 32] = blk
    put("cw1p", t)
    t = np.zeros((96, 192), f32)
    for ky in range(3):
        for kx in range(3):
            t[ky * 32:(ky + 1) * 32, kx * 64:(kx + 1) * 64] = cw[1][:, :, ky, kx].T
    put("cw2p", t)
    ta = np.zeros((128, 384), f32)
    tb = np.zeros((64, 384), f32)
    for kx in range(3):
        for ky in range(2):
            ta[ky * 64:(ky + 1) * 64, kx * 128:(kx + 1) * 128] = cw[2][:, :, ky, kx].T
        tb[:, kx * 128:(kx + 1) * 128] = cw[2][:, :, 2, kx].T
    put("cw3ap", ta)
    put("cw3bp", tb)
    t = np.zeros((128, 1152), f32)
    for k in range(9):
        ky, kx = divmod(k, 3)
        t[:, k * 128:(k + 1) * 128] = cw[3][:, :, ky, kx].T
    put("cw4p", t)
    put("awr", np.asarray(inputs["aw"], f32)[:, :, 0])

    wpkf = np.zeros((128, 8), f32)
    wpkf[:, 0] = np.tile(cb[0], 4)
    wpkf[:, 1] = np.tile(cb[1], 2)
    wpkf[:, 2] = cb[2]
    wpkf[:, 3] = cb[3]
    wpkf[:, 4] = np.asarray(inputs["ab"], f32).reshape(128)
    return {"wpk": wpk.astype(BF), "wpkf": wpkf}


def _host_shard(inputs, c):
    f32 = np.float32
    E0 = EPC * c
    ew = [np.asarray(inputs[f"ew{i+1}"], f32)[E0:E0 + EPC] for i in range(5)]
    eb = [np.asarray(inputs[f"eb{i+1}"], f32)[E0:E0 + EPC] for i in range(5)]
    km = lambda a: np.ascontiguousarray(a.transpose(1, 0, 2))
    aug = lambda w, b: np.concatenate([w, b[None]], 0)

    epk = np.zeros((128, EPK_COLS), f32)

    def put(name, arr):
        r, o, cc = EPK_LAYOUT[name]
        assert arr.shape == (r, cc), (name, arr.shape)
        epk[0:r, o:o + cc] = arr

    for li, pre in ((0, "e1"), (1, "e2")):
        w, b = km(ew[li]), eb[li]
        put(pre + "AA", w[0:128, :, 0:128].reshape(128, -1))
        put(pre + "AB", w[0:128, :, 128:196].reshape(128, -1))
        put(pre + "BA", aug(w[128:196, :, 0:128], b[:, 0:128]).reshape(69, -1))
        put(pre + "BB", aug(w[128:196, :, 128:196], b[:, 128:196]).reshape(69, -1))
    w3 = km(ew[2])
    put("e3A", w3[0:128].reshape(128, -1))
    put("e3B", aug(w3[128:196], eb[2]).reshape(69, -1))
    put("e4", aug(km(ew[3]), eb[3]).reshape(99, -1))
    t = np.zeros((64, 256), f32)
    for p in range(8):
        for e2 in range(2):
            e = 2 * p + e2
            rb, cb_ = 32 * e2, p * 32 + e2 * 16
            t[rb:rb + 24, cb_:cb_ + 16] = ew[4][e]
            t[rb + 24, cb_:cb_ + 16] = eb[4][e]
    put("e5p", t)

    fw1 = np.asarray(inputs["fw1"], f32)
    t = np.zeros((128, 4096), f32)
    for g in range(2):
        for row in range(128):
            pp, r32 = divmod(row, 32)
            e2, o = divmod(r32, 16)
            el = (g * 4 + pp) * 2 + e2
            t[row, g * 2048:g * 2048 + FIN[1]] = fw1[(E0 + el) * 16 + o]
    put("fw1s", t)
    put("fb1d8", np.pad(np.asarray(inputs["fb1"], f32) / 8,
                        (0, 2048 - FIN[1])).reshape(1, 2048))
    fw2 = np.asarray(inputs["fw2"], f32)
    t = np.zeros((128, 4096), f32)
    for kc in range(2):
        m0 = 256 * c + kc * 128
        n = max(0, min(128, FIN[1] - m0))
        if n > 0:
            t[0:n, kc * 2048:kc * 2048 + FIN[2]] = fw2[m0:m0 + n]
    put("fw2s", t)
    put("fb2d8", np.pad(np.asarray(inputs["fb2"], f32) / 8,
                        (0, 2048 - FIN[2])).reshape(1, 2048))
    fw3 = np.asarray(inputs["fw3"], f32)
    t = np.zeros((128, 106), f32)
    for kc in range(2):
        m0 = 256 * c + kc * 128
        n = max(0, min(128, FIN[2] - m0))
        if n > 0:
            t[0:n, kc * 53:kc * 53 + 53] = fw3[m0:m0 + n]
    put("fw3s", t)
    put("fb3d8", (np.asarray(inputs["fb3"], f32) / 8).reshape(1, 53))
    put("onesr", np.ones((1, 512), f32))
    t5f = np.zeros((64, 256), f32)
    t5f[24, :] = 1
    t5f[56, :] = 1
    put("t5f", t5f)
    return {"epk": epk.astype(BF)}


def _in_maps(inputs):
    shared = _host_shared(inputs)
    x = np.asarray(inputs["x"], np.float32)
    maps = []
    for c in range(NCORES):
        m = dict(shared)
        m.update(_host_shard(inputs, c))
        xpl = np.zeros((BL, 3, 228, 228), np.float32)
        xpl[:, :, 1:225, 1:225] = x[c * BL:(c + 1) * BL]
        x9 = np.empty((9, 12, 226, 226), np.float32)
        for ky in range(3):
            for kx in range(3):
                x9[ky * 3 + kx] = xpl[:, :, ky:ky + 226, kx:kx + 226].reshape(12, 226, 226)
        m["xpad9"] = x9.reshape(-1).astype(BF)
        maps.append(m)
    return maps


def kernel(**inputs):
    global _BUILT
    if _BUILT is None:
        _BUILT = _build()
    res = run_bass_kernel_spmd(_BUILT, _in_maps(inputs), list(range(NCORES)))
    return np.sum([res.results[c]["y"] for c in range(NCORES)], axis=0,
                  dtype=np.float32)
